# revision 1
# baseline (speedup 1.0000x reference)
"""DepthSensitiveLoss on 8 Trainium2 NeuronCores (Bass/Tile).

Data-parallel over the batch dim: each core processes 1024 rows of the
8192x4096 inputs, producing per-row wbce partial sums and per-row max
streaks; the host combines the 8x[128,16] partials into the scalar loss.

Per [128, 4096] tile (full rows in the free dim), with x = y_pred + y_true - 1:
  bce      = -ln(|x| + EPS)            (y_true is exactly 0/1)
  correct  = |x| > 0.5                 (equiv. to (y_pred > 0.5) == y_true)
  streak_t = correct_t * (streak_{t-1} + 1)   -> tensor_tensor_scan
"""

import numpy as np

B, N = 8192, 4096
NCORES = 8
ROWS_PER_CORE = B // NCORES  # 1024
P = 128
T = ROWS_PER_CORE // P  # 8 tiles per core
CH = 2  # compute chunks per tile (DMAs stay full-width)
W = N // CH
ALPHA = 0.5
EPS = 1e-6

_cached_nc = None
LAST_RESULTS = None  # stash for test harness introspection


def _legalize_waits(bir: bytes) -> bytes:
    """Spill extra sync waits onto NOPs: the walrus codegen here encodes at
    most 1 sync wait per instruction (2 for EventSemaphore), but Tile attaches
    full wait lists (e.g. on the kernel-tail Drain). Hoisting the surplus onto
    same-engine NOPs immediately before the instruction is semantically
    identical: the engine blocks on all sems either way before executing it."""
    import json

    j = json.loads(bir)
    counter = [0]

    def fix_block(insts):
        out = []
        for inst in insts:
            si = inst.get("sync_info")
            if si:
                ow = si.get("on_wait") or []
                cap = 2 if inst.get("opcode") == "EventSemaphore" else 1
                if len(ow) > cap:
                    for w in ow[:-cap]:
                        counter[0] += 1
                        out.append(
                            {
                                "debug": inst.get("debug", 0),
                                "engine": inst["engine"],
                                "ins": [],
                                "name": f"LegalWait-{counter[0]}",
                                "opcode": "NoOp",
                                "outs": [],
                                "sync_info": {"on_update": [], "on_wait": [w]},
                            }
                        )
                    si["on_wait"] = ow[-cap:]
            out.append(inst)
        return out

    def walk(obj):
        if isinstance(obj, dict):
            if isinstance(obj.get("instructions"), list):
                obj["instructions"] = fix_block(obj["instructions"])
            for v in obj.values():
                walk(v)
        elif isinstance(obj, list):
            for v in obj:
                walk(v)

    walk(j)
    return json.dumps(j).encode()


def _build(reps: int = 1, mode: str = "full"):
    import concourse.bass as bass
    import concourse.mybir as mybir
    import concourse.tile as tile

    Op = mybir.AluOpType
    Act = mybir.ActivationFunctionType
    f32 = mybir.dt.float32
    bf16 = mybir.dt.bfloat16

    nc = bass.Bass()
    yp = nc.dram_tensor("y_pred", [ROWS_PER_CORE, N], f32, kind="ExternalInput")
    yt = nc.dram_tensor("y_true", [ROWS_PER_CORE, N], f32, kind="ExternalInput")
    dw = nc.dram_tensor("depth_weights", [ROWS_PER_CORE, N], f32, kind="ExternalInput")
    # tile-major layout: each tile's [P, 2*CH] block is contiguous in DRAM,
    # so the per-tile store is one dense 2KB write instead of 128 scattered
    # 16B pieces across the row-major span.
    out = nc.dram_tensor("partials", [T * P, 2 * CH], f32, kind="ExternalOutput")
    out_t = out.rearrange("(t p) c -> t p c", p=P)

    yp_t = yp.rearrange("(t p) n -> t p n", p=P)
    yt_t = yt.rearrange("(t p) n -> t p n", p=P)
    dw_t = dw.rearrange("(t p) n -> t p n", p=P)

    with tile.TileContext(nc) as tc:
        with (
            tc.tile_pool(name="biga", bufs=3) as pool_a,
            tc.tile_pool(name="bigb", bufs=3) as pool_b,
            tc.tile_pool(name="bigc", bufs=3) as pool_c,
            tc.tile_pool(name="bigr", bufs=2) as pool_r,
            tc.tile_pool(name="small", bufs=T) as small,
            tc.tile_pool(name="consts", bufs=1) as consts,
        ):
            bias = consts.tile([P, 3], f32)
            nc.vector.memset(bias[:, 0:1], 0.0)
            nc.vector.memset(bias[:, 1:2], EPS)
            nc.vector.memset(bias[:, 2:3], -1.0)

            for t in [tt for _ in range(reps) for tt in range(T)]:
                ch, wd = CH, W
                a = pool_a.tile([P, N], f32, tag="a")  # y_pred -> +y_true -> |x|
                b = pool_b.tile([P, N], f32, tag="b")  # y_true
                c = pool_c.tile([P, N], f32, tag="c")  # depth_weights -> wbce product
                if mode == "dmaonly2":
                    # balance the two HWDGE rings: 1.5 tensors each per tile
                    e0, e1 = (nc.sync, nc.scalar) if t % 2 == 0 else (nc.scalar, nc.sync)
                    e0.dma_start(a[:], yp_t[t, :, :])
                    e1.dma_start(b[:], yt_t[t, :, :])
                    e0.dma_start(c[:, : N // 2], dw_t[t, :, : N // 2])
                    e1.dma_start(c[:, N // 2 :], dw_t[t, :, N // 2 :])
                else:
                    nc.sync.dma_start(a[:], yp_t[t, :, :])
                    nc.scalar.dma_start(b[:], yt_t[t, :, :])
                    nc.sync.dma_start(c[:], dw_t[t, :, :])

                s = small.tile([P, 2 * CH], f32, tag="s")

                if mode in ("dmaonly", "dmaonly2"):
                    nc.vector.memset(s[:], 0.0)
                    nc.sync.dma_start(out_t[t, :, :], s[:])
                    continue

                r = pool_r.tile([P, N], f32, tag="r")  # correct -> streaks
                for k in range(ch):
                    w = slice(k * wd, (k + 1) * wd)
                    # x = (y_pred - 1) + y_true       [DVE stt, in-place a]
                    # (the Pool engine's 2-input rate measured far below spec
                    # and it headed every tile's dependency chain; all-DVE
                    # measured faster than any Pool split)
                    if mode != "nopool":
                        nc.vector.scalar_tensor_tensor(
                            a[:, w], a[:, w], 1.0, b[:, w], Op.subtract, Op.add
                        )
                    if mode != "noact":
                        # |x|                          [ACT, in-place a]
                        nc.scalar.activation(
                            a[:, w], a[:, w], Act.Abs, bias=bias[:, 0:1]
                        )
                    # correct = |x| > 0.5   [DVE 1-input tensor_scalar -> r]
                    # (1-input fp32 tensor_scalar hits the DVE 2x mode; the
                    # 2-input stt form (yp>0.5)==yt measured ~24us/pass slower,
                    # and abs_max to fold the abs fails is_valid_aluop here)
                    nc.vector.tensor_scalar(r[:, w], a[:, w], 0.5, None, Op.is_gt)
                    if mode != "noact":
                        # ln(|x| + EPS)                [ACT, in-place a]
                        nc.scalar.activation(
                            a[:, w], a[:, w], Act.Ln, bias=bias[:, 1:2]
                        )
                    # streak scan: s_j = correct_j*(s_{j-1}+1)  [DVE, in-place r]
                    # chunks chain through the previous chunk's last column
                    init = 0.0 if k == 0 else r[:, k * wd - 1 : k * wd]
                    if mode != "noscan":
                        nc.vector.tensor_tensor_scan(
                            r[:, w], r[:, w], r[:, w], init, Op.mult, Op.add
                        )
                    # per-row max streak for this chunk   [DVE]
                    nc.vector.tensor_reduce(
                        s[:, 2 * k + 1 : 2 * k + 2], r[:, w],
                        mybir.AxisListType.X, Op.max,
                    )
                # wbce row sums = sum((ln * -1) * dw)  [DVE, fused accum]
                for k in range(ch):
                    w = slice(k * wd, (k + 1) * wd)
                    nc.vector.scalar_tensor_tensor(
                        c[:, w], a[:, w], -1.0, c[:, w], Op.mult, Op.mult,
                        accum_out=s[:, 2 * k : 2 * k + 1],
                    )
                nc.sync.dma_start(out_t[t, :, :], s[:])

    _orig_to_json = nc.to_json_bytes
    nc.to_json_bytes = lambda: _legalize_waits(_orig_to_json())
    return nc


def kernel(y_pred, y_true, depth_weights):
    global _cached_nc, LAST_RESULTS
    import os

    # The axon client here has no NTFF profile hook; a BASS_TRACE=1 in the
    # environment would crash run_bass_kernel_spmd on a missing import.
    os.environ["BASS_NEVER_TRACE"] = "1"

    from concourse.bass_utils import run_bass_kernel_spmd

    if _cached_nc is None:
        _cached_nc = _build()
    nc = _cached_nc

    y_pred = np.ascontiguousarray(np.asarray(y_pred, dtype=np.float32))
    y_true = np.ascontiguousarray(np.asarray(y_true, dtype=np.float32))
    depth_weights = np.ascontiguousarray(np.asarray(depth_weights, dtype=np.float32))

    in_maps = []
    for i in range(NCORES):
        r0, r1 = i * ROWS_PER_CORE, (i + 1) * ROWS_PER_CORE
        in_maps.append(
            {
                "y_pred": y_pred[r0:r1],
                "y_true": y_true[r0:r1],
                "depth_weights": depth_weights[r0:r1],
            }
        )

    res = run_bass_kernel_spmd(nc, in_maps, core_ids=list(range(NCORES)))
    LAST_RESULTS = res

    parts = np.stack([r["partials"] for r in res.results])  # [8, T*P, 2*CH]
    wbce_sum = parts[:, :, 0::2].sum(dtype=np.float64)
    streak_sum = parts[:, :, 1::2].max(axis=2).sum(dtype=np.float64)
    wbce = wbce_sum / (B * N)
    cwl = 1.0 - streak_sum / (N * B)
    return np.asarray(ALPHA * wbce + (1.0 - ALPHA) * cwl, dtype=np.float32)

